# revision 12
# baseline (speedup 1.0000x reference)
"""Cross-attention layer on 8 TRN2 NeuronCores.

Sharding: core i -> (batch b = i//2, head-group g = i%2); each core computes
its head-group's contribution to out[b] through Wo; the host sums the two
partial products per batch (row-split of Wo => partial-sum reduction).

Device kernel works in transposed layout ([channels, tokens]) so the softmax
reduction is along the matmul free axis:
  Q^T = Wq_g^T x^T, K^T = Wk_g^T ctx^T, V' = [ctx Wv_g | ones(64)]
  scores^T_h = K_h Q_h^T  (contraction over head_dim=64)
  E = exp(scores^T/32) * mask^T      (no max subtraction; |scores/32| ~ 1.5)
  U = V'^T E  (per s-tile accumulation; rows 64..127 are 64 identical
               copies of the softmax denominator -- the ones block makes
               the PE broadcast the denominator for free)
  O^T = U[0:64] * exp(-ln(U[64:128]))  (ln+exp on ACT -- the activation
               table is pinned to natural_log_exp_and_others so Exp, Ln
               and Copy never force a table reload)
  out_partial = O^T^T Wo_g           (host adds core pairs)

The whole kernel is one software pipeline: scores+exp for units
(head, t-chunk) run ~5 units ahead of their PV matmuls, with the early
units' exp/mask work overlapped into the projection phase, so the PE (the
critical engine at ~110us of matmul work) rarely waits on the ACT exp
stream and stays at its max p-state.

Dtype split: x/ctx/Wq/Wk/Wv, Q^T/K^T/V', probs, O^T and Wo run in bf16;
score/U accumulation in fp32 PSUM; final out in fp32.
"""

import os
import numpy as np
import ml_dtypes

import concourse.mybir as mybir
from concourse import bacc
import concourse.tile as tile
from concourse.bass_utils import run_bass_kernel_spmd

B, T, TC = 4, 1024, 1024
C, CTX_C, H = 1024, 1024, 16
HD = C // H            # 64
P = 128
NCORES = 8
HG = 2                 # head groups
HPG = H // HG          # 8 heads per core
CG = HPG * HD          # 512 channels per group
NT = 512               # matmul free-dim chunk
KO = C // P            # 8 contraction tiles for projections
MQ = CG // P           # 4 partition-tiles of Q^T/K^T
SO = TC // P           # 8 s-tiles
T2 = T // NT           # 2 t-chunks
KP = CG // P           # 4 contraction tiles for the out projection
NU = HPG * T2          # 16 pipeline units
F32 = mybir.dt.float32
BF16 = mybir.dt.bfloat16
ALU = mybir.AluOpType
ACTF = mybir.ActivationFunctionType

_CACHED_NC = None
_PINNED_TABLE = "natural_log_exp_and_others"


class _PinnedBacc(bacc.Bacc):
    """Bacc whose act-table selection is pinned to one combined table.

    The rust insert_act_table_loads pass picks, per activation, some table
    containing the required function; with Exp and Ln in play it flip-flops
    between exp-only and ln-only tables (1.3us per reload).  Stripping
    Exp/Ln/Copy/Identity from every other table entry (ids keep their
    positions, so the runtime table contents are unchanged) forces every
    activation onto the one table that serves all three -> a single load.
    """

    def insert_act_table_loads(self):
        has_activation = any(
            isinstance(i, mybir.InstActivation)
            for b in self.main_func.blocks
            for i in b.instructions
        )
        if not has_activation:
            return
        from concourse.hw_specs import get_activation_tables
        pinned_funcs = {ACTF.Exp, ACTF.Ln, ACTF.Copy, ACTF.Identity}
        tables = []
        for name, funcs in get_activation_tables(self.m.arch).items():
            if name != _PINNED_TABLE:
                funcs = set(funcs) - pinned_funcs
            tables.append((name, funcs))
        bacc._bass_rust.insert_act_table_loads(self, tables)


def _ensure_ntff_hook():
    """Register the axon NTFF profiling hook if the image's antenv lacks it."""
    try:
        from antenv.axon_hooks import get_axon_ntff_profile_hook  # noqa: F401
        return
    except ImportError:
        pass
    import sys
    import types
    try:
        from trn_agent_boot.trn_boot import _ntff_profile_via_ctypes
        hook = _ntff_profile_via_ctypes("/opt/axon/libaxon_pjrt.so")
    except Exception:
        hook = None
    mod = types.ModuleType("antenv.axon_hooks")
    mod.get_axon_ntff_profile_hook = lambda: hook
    mod.set_axon_ntff_profile_hook = lambda h: None
    sys.modules["antenv.axon_hooks"] = mod
    import antenv
    antenv.axon_hooks = mod


def _hp(h):
    """Partition slice of local head h inside a [128, MQ, ...] channel tile."""
    lo = (h % 2) * HD
    return slice(lo, lo + HD)


def _build_program():
    nc = _PinnedBacc("TRN2", target_bir_lowering=False, debug=False,
                     num_devices=NCORES)
    xT = nc.dram_tensor("xT", [C, T], BF16, kind="ExternalInput").ap()
    ctxT = nc.dram_tensor("ctxT", [CTX_C, TC], BF16, kind="ExternalInput").ap()
    maskT = nc.dram_tensor("maskT", [TC, T], BF16, kind="ExternalInput").ap()
    wq = nc.dram_tensor("wq", [C, CG], BF16, kind="ExternalInput").ap()
    wk = nc.dram_tensor("wk", [CTX_C, CG], BF16, kind="ExternalInput").ap()
    wv = nc.dram_tensor("wv", [CTX_C, CG], BF16, kind="ExternalInput").ap()
    wo = nc.dram_tensor("wo", [CG, C], BF16, kind="ExternalInput").ap()
    out = nc.dram_tensor("out", [T, C], F32, kind="ExternalOutput").ap()

    # unit u: head h = u % HPG, t-chunk t2 = u // HPG  (t2-major so the
    # out projection of the first t-half overlaps the second half's attention)
    units = [(u % HPG, u // HPG) for u in range(NU)]

    with tile.TileContext(nc) as tc:
        with (
            tc.tile_pool(name="persist", bufs=1) as persist,
            tc.tile_pool(name="etp", bufs=6) as etp,
            tc.tile_pool(name="work", bufs=3) as work,
            tc.tile_pool(name="psmm", bufs=2, space="PSUM") as psmm,
            tc.tile_pool(name="pssc", bufs=2, space="PSUM") as pssc,
            tc.tile_pool(name="psu", bufs=2, space="PSUM") as psu_pool,
        ):
            qt_sb = persist.tile([P, MQ, T], BF16)            # Q^T [(h,d), t]
            kt_sb = persist.tile([P, MQ, TC], BF16)           # K^T [(h,d), s]
            vp_sb = persist.tile([P, SO, HPG, P], BF16)       # V' + ones blk
            mask_sb = persist.tile([P, SO, T], BF16)          # mask^T
            ot_sb = persist.tile([P, KP, T], BF16)            # O^T normalized
            wo_sb = persist.tile([P, KP, C], BF16)
            xT_sb = persist.tile([P, KO, T], BF16)
            ctxT_sb = persist.tile([P, KO, TC], BF16)
            wq_sb = persist.tile([P, KO, CG], BF16)
            wk_sb = persist.tile([P, KO, CG], BF16)
            wv_sb = persist.tile([P, KO, CG], BF16)

            nc.gpsimd.memset(vp_sb[:, :, :, HD:P], 1.0)

            xT_r = xT.rearrange("(ko p) t -> p ko t", p=P)
            ctxT_r = ctxT.rearrange("(h ko p) t -> p h ko t", p=P, h=2)
            ctxT_s = ctxT_sb.rearrange("p (h ko) t -> p h ko t", h=2)
            wq_r = wq.rearrange("(ko p) m -> p ko m", p=P)
            # kc0 of wq/xT land first so the very first Q matmul (and the PE
            # p-state ramp) starts ~6us earlier than a monolithic transfer
            nc.sync.dma_start(wq_sb[:, 0], wq_r[:, 0])
            nc.sync.dma_start(xT_sb[:, 0], xT_r[:, 0])
            nc.sync.dma_start(wq_sb[:, 1:], wq_r[:, 1:])
            nc.sync.dma_start(xT_sb[:, 1:4], xT_r[:, 1:4])
            nc.sync.dma_start(xT_sb[:, 4:], xT_r[:, 4:])
            nc.sync.dma_start(wk_sb, wk.rearrange("(ko p) m -> p ko m", p=P))
            for h in range(2):
                nc.sync.dma_start(ctxT_s[:, h], ctxT_r[:, h])
            nc.sync.dma_start(mask_sb,
                              maskT.rearrange("(so p) t -> p so t", p=P))
            nc.sync.dma_start(wv_sb, wv.rearrange("(ko p) m -> p ko m", p=P))
            nc.sync.dma_start(wo_sb, wo.rearrange("(ko p) n -> p ko n", p=P))

            def proj_q(m, act_evict=False):
                for t2 in range(T2):
                    ps = psmm.tile([P, NT], F32, tag="mm512")
                    for kc in range(KO):
                        nc.tensor.matmul(
                            ps, wq_sb[:, kc, m * P:(m + 1) * P],
                            xT_sb[:, kc, t2 * NT:(t2 + 1) * NT],
                            start=(kc == 0), stop=(kc == KO - 1))
                    dst = qt_sb[:, m, t2 * NT:(t2 + 1) * NT]
                    if act_evict:   # ACT is idle pre-softmax; DVE would
                        nc.scalar.activation(dst, ps, ACTF.Copy)
                    else:           # stall behind the mask-DMA-gated mults
                        nc.vector.tensor_copy(dst, ps)

            def proj_k(m, act_evict=False):
                for s2 in range(T2):
                    ps = psmm.tile([P, NT], F32, tag="mm512")
                    for kc in range(KO):
                        nc.tensor.matmul(
                            ps, wk_sb[:, kc, m * P:(m + 1) * P],
                            ctxT_sb[:, kc, s2 * NT:(s2 + 1) * NT],
                            start=(kc == 0), stop=(kc == KO - 1))
                    dst = kt_sb[:, m, s2 * NT:(s2 + 1) * NT]
                    if act_evict:
                        nc.scalar.activation(dst, ps, ACTF.Copy)
                    else:
                        nc.vector.tensor_copy(dst, ps)

            def proj_v():
                for so in range(SO):     # V = ctx Wv  (natural layout)
                    ps = psmm.tile([P, NT], F32, tag="mm512")
                    for kc in range(KO):
                        nc.tensor.matmul(
                            ps, ctxT_sb[:, kc, so * P:(so + 1) * P],
                            wv_sb[:, kc, :],
                            start=(kc == 0), stop=(kc == KO - 1))
                    nc.vector.tensor_copy(
                        vp_sb[:, so, :, 0:HD],
                        ps.rearrange("p (h d) -> p h d", h=HPG))

            def scores_unit(u):
                h, t2 = units[u]
                et = etp.tile([P, SO, NT], BF16, tag="exp")
                for j in range(SO // 2):   # s-tile pairs share a 2-bank psum
                    ps = pssc.tile([P, 2 * NT], F32, tag="ps_sc")
                    for i in range(2):
                        so = 2 * j + i
                        nc.tensor.matmul(
                            ps[:, i * NT:(i + 1) * NT],
                            kt_sb[_hp(h), h // 2, so * P:(so + 1) * P],
                            qt_sb[_hp(h), h // 2, t2 * NT:(t2 + 1) * NT],
                            start=True, stop=True)
                    nc.scalar.activation(
                        et[:, 2 * j:2 * j + 2, :].rearrange("p a b -> p (a b)"),
                        ps, ACTF.Exp, scale=1.0 / 32.0)
                    nc.vector.tensor_tensor(
                        et[:, 2 * j:2 * j + 2, :],
                        et[:, 2 * j:2 * j + 2, :],
                        mask_sb[:, 2 * j:2 * j + 2, t2 * NT:(t2 + 1) * NT],
                        ALU.mult)
                return et

            ub_state = {}

            def pv_unit(u, et):
                h, t2 = units[u]
                psu = psu_pool.tile([P, NT], F32, tag="ps_u")
                for so in range(SO):
                    nc.tensor.matmul(
                        psu, vp_sb[:, so, h, :], et[:, so, :],
                        start=(so == 0), stop=(so == SO - 1))
                # two quick DVE half-copies (both landing at partition base 0,
                # as the final tensor_tensor requires) release the psum bank;
                # the ln/exp reciprocal runs on the SBUF copies batched over
                # 4 units (2048-wide frees amortize the per-op ACT overhead)
                if u % 4 == 0:
                    ub_state["a"] = work.tile([HD, 4, NT], BF16, tag="ua",
                                              name="ua", bufs=2)
                    ub_state["b"] = work.tile([HD, 4, NT], BF16, tag="ub",
                                              name="ub", bufs=2)
                ua, ub = ub_state["a"], ub_state["b"]
                nc.vector.tensor_copy(ua[:, u % 4, :], psu[0:HD, :])
                nc.vector.tensor_copy(ub[:, u % 4, :], psu[HD:P, :])
                if u % 4 == 3:
                    den = ub.rearrange("p a b -> p (a b)")
                    rec = work.tile([HD, 4 * NT], BF16, tag="rec",
                                    name="rec", bufs=1)
                    nc.scalar.activation(rec, den, ACTF.Ln)
                    nc.scalar.activation(den, rec, ACTF.Exp, scale=-1.0)
                    for ui in range(u - 3, u + 1):
                        hi, ti = units[ui]
                        nc.vector.tensor_tensor(
                            ot_sb[_hp(hi), hi // 2, ti * NT:(ti + 1) * NT],
                            ua[:, ui % 4, :], ub[:, ui % 4, :],
                            ALU.mult)

            def dproj(i):
                tm, c2 = i // 2, i % 2
                # tail groups (i>=8) alternate psum pools (psu is free after
                # the last pv) and split evictions ACT/DVE so the 2-buf psmm
                # ring stops serializing the out projection
                if i >= 8 and i % 2 == 1:
                    ps = psu_pool.tile([P, NT], F32, tag="ps_u")
                else:
                    ps = psmm.tile([P, NT], F32, tag="mm512")
                for kp in range(KP):
                    nc.tensor.matmul(
                        ps, ot_sb[:, kp, tm * P:(tm + 1) * P],
                        wo_sb[:, kp, c2 * NT:(c2 + 1) * NT],
                        start=(kp == 0), stop=(kp == KP - 1))
                o_sb = work.tile([P, NT], F32, tag="out")
                if i >= 8 and i % 2 == 0:
                    nc.scalar.activation(o_sb, ps, ACTF.Copy)
                else:
                    nc.vector.tensor_copy(o_sb, ps)
                nc.sync.dma_start(
                    out[tm * P:(tm + 1) * P, c2 * NT:(c2 + 1) * NT], o_sb)

            # ---- software pipeline ----
            et_q = {}
            proj_q(0, act_evict=True); proj_k(0, act_evict=True)
            et_q[0] = scores_unit(0)           # exp stream starts early
            et_q[1] = scores_unit(1)
            proj_q(1, act_evict=True); proj_k(1)
            et_q[2] = scores_unit(2)
            et_q[3] = scores_unit(3)
            proj_q(2); proj_k(2)
            et_q[4] = scores_unit(4)
            et_q[5] = scores_unit(5)
            proj_v()
            proj_q(3); proj_k(3)
            pv_unit(0, et_q.pop(0))
            et_q[6] = scores_unit(6)
            pv_unit(1, et_q.pop(1))
            et_q[7] = scores_unit(7)
            pv_unit(2, et_q.pop(2))
            for u in range(3, 11):
                et_q[u + 5] = scores_unit(u + 5)
                pv_unit(u, et_q.pop(u))
                if u >= 8:                     # t2=0 done: overlap out proj
                    dproj(u - 8)
            for u in range(11, 16):
                pv_unit(u, et_q.pop(u))
                dproj(u - 8)
            for i in range(8, 16):
                dproj(i)
    nc.compile()
    return nc


def _get_program():
    global _CACHED_NC
    if _CACHED_NC is None:
        _CACHED_NC = _build_program()
    return _CACHED_NC


def kernel(x, context, attn_mask, Wq, Wk, Wv, Wo):
    x = np.asarray(x, dtype=np.float32)
    context = np.asarray(context, dtype=np.float32)
    attn_mask = np.asarray(attn_mask)
    Wq = np.asarray(Wq, dtype=np.float32)
    Wk = np.asarray(Wk, dtype=np.float32)
    Wv = np.asarray(Wv, dtype=np.float32)
    Wo = np.asarray(Wo, dtype=np.float32)

    nc = _get_program()
    bf = ml_dtypes.bfloat16
    in_maps = []
    for i in range(NCORES):
        b, g = i // 2, i % 2
        cs = slice(g * CG, (g + 1) * CG)
        in_maps.append({
            "xT": np.ascontiguousarray(x[b].T).astype(bf),
            "ctxT": np.ascontiguousarray(context[b].T).astype(bf),
            "maskT": np.ascontiguousarray(attn_mask[b, 0].T).astype(bf),
            "wq": np.ascontiguousarray(Wq[:, cs]).astype(bf),
            "wk": np.ascontiguousarray(Wk[:, cs]).astype(bf),
            "wv": np.ascontiguousarray(Wv[:, cs]).astype(bf),
            "wo": np.ascontiguousarray(Wo[cs, :]).astype(bf),
        })

    profile = os.environ.get("KERNEL_PROFILE", "0") == "1"
    if profile:
        _ensure_ntff_hook()
    res = run_bass_kernel_spmd(
        nc, in_maps, list(range(NCORES)),
        trace=profile, trace_cores=[0] if profile else None)
    if profile:
        kernel.last_exec_time_ns = res.exec_time_ns
        kernel.last_trace = res.instructions_and_trace

    out = np.empty((B, T, C), dtype=np.float32)
    for b in range(B):
        out[b] = res.results[2 * b]["out"] + res.results[2 * b + 1]["out"]
    return out
